# revision 1
# baseline (speedup 1.0000x reference)
"""RGCN (2-layer, mean-aggregation) Bass kernel for one TRN2 chip (8 NeuronCores).

Strategy (edge/dst-sharded, aggregate-then-transform):
  - Nodes are block-partitioned across the 8 cores (12500/core). Edges live on
    their *dst*-owner core, so all aggregation is core-local (no reduce).
  - x is replicated (bf16) in every core's HBM. Per edge: dma_gather x[src]
    (256B rows), scale by inv_deg (precomputed on host), dma_scatter_add into
    per-relation accumulators A_r[dst_local] (bf16, zero-initialized as
    ExternalOutputs).
  - Transform phase: out[dst] = relu(sum_r A_r @ W_r + x_local @ root + b),
    computed as 128x128 PSUM tiles; A_r^T tiles come in via HW DMA-transpose
    (bf16), weights are bf16, accumulation is f32 in PSUM.
  - Between layers, a single small AllGather (bf16, 3.2MB/rank) replicates the
    new features; layer-1's replication is free (host pre-stages x_rep).
  - int16 gather/scatter indices are kept in range by grouping edges by
    (src block, relation-pair): gather tables are the 12544-row node blocks of
    the replicated x; scatter targets are [2*12544, 128] relation-pair tables.
"""

import numpy as np
import ml_dtypes

import concourse.tile as tile
from concourse import bass, bacc, mybir
from concourse.bass_utils import run_bass_kernel_spmd

BF16 = mybir.dt.bfloat16
F32 = mybir.dt.float32
I16 = mybir.dt.int16
bf16 = ml_dtypes.bfloat16
import os
# max indices per gather/scatter call: single_packet=True caps at 1024
# (multi-call ring pressure), single_packet=False caps at 1920 (<2048 field)
SINGLE_PACKET = os.environ.get("K_SP", "0") == "1"
MAXC = 1024 if SINGLE_PACKET else 1920

# ----------------------------------------------------------------------------
# Problem constants (full-size; mini configs for sim tests override these)
# ----------------------------------------------------------------------------
FULL = dict(N=100000, E=1000000, D=128, R=8, C=8)


def derive(cfg):
    N, D, R, C = cfg["N"], cfg["D"], cfg["R"], cfg["C"]
    NL = N // C                      # owned nodes per core
    NT = (NL + 128) // 128           # node tiles per core (>= 1 dustbin row)
    NLP = NT * 128                   # padded rows per block
    RP = R // 2                      # relation-pair tables
    # transform chunk size (tiles per DMA-transpose batch)
    TCH = 14 if NT % 14 == 0 else (7 if NT % 7 == 0 else (3 if NT % 3 == 0 else 1))
    return NL, NT, NLP, RP, TCH


# ----------------------------------------------------------------------------
# Host-side preprocessing
# ----------------------------------------------------------------------------
def host_prep(x, edge_index, edge_type, cfg):
    """Sort/pad edges, build wrapped int16 index tiles and scale tiles."""
    N, E, D, R, C = cfg["N"], cfg["E"], cfg["D"], cfg["R"], cfg["C"]
    NL, NT, NLP, RP, TCH = derive(cfg)

    src = np.asarray(edge_index[0], dtype=np.int64)
    dst = np.asarray(edge_index[1], dtype=np.int64)
    et = np.asarray(edge_type, dtype=np.int64)

    # mean-normalization per (relation, dst), computed on host (graph-only)
    deg = np.zeros((R, N), np.float32)
    np.add.at(deg, (et, dst), 1.0)
    inv = np.where(deg > 0, 1.0 / np.maximum(deg, 1.0), 0.0).astype(np.float32)
    scale_e = inv[et, dst]

    core = dst // NL
    blk = src // NL
    rp = et // 2
    unit = blk * RP + rp                      # unit id within a core
    NU = C * RP                               # units per core
    # scatter index within the relation-pair table
    sidx_val = (et % 2) * NLP + (dst % NL)
    gidx_val = src % NL
    DUSTBIN = NL                              # pad rows add 0.0 to this row

    # The HW scatter-add loses updates when the same destination row appears
    # more than once in one call (concurrent RMW descriptors). Within each
    # (core, unit) group we therefore rank duplicate sidx occurrences and
    # emit one scatter sub-call per rank: indices are unique within a call,
    # and calls targeting the same table are serialized.
    order = np.lexsort((sidx_val, unit, core))
    key_core, key_unit, key_sidx = core[order], unit[order], sidx_val[order]
    new_run = np.ones(len(order), bool)
    new_run[1:] = (
        (key_core[1:] != key_core[:-1])
        | (key_unit[1:] != key_unit[:-1])
        | (key_sidx[1:] != key_sidx[:-1])
    )
    run_id = np.cumsum(new_run) - 1
    run_starts = np.flatnonzero(new_run)
    rank = np.arange(len(order)) - run_starts[run_id]

    # final order: (core, unit, rank, sidx)
    order2 = order[np.lexsort((key_sidx, rank, key_unit, key_core))]
    rank2 = rank[np.lexsort((key_sidx, rank, key_unit, key_core))]
    src_o, core_o, unit_o = gidx_val[order2], core[order2], unit[order2]
    sidx_o, scale_o = sidx_val[order2], scale_e[order2]

    NRANK = int(rank.max()) + 1
    # per-(core, unit, rank) counts; span size = max over cores, padded to 128
    counts = np.zeros((C, NU, NRANK), np.int64)
    np.add.at(counts, (core_o, unit_o, rank2), 1)
    spans = (np.ceil(counts.max(axis=0) / 128).astype(np.int64) * 128)  # [NU, NRANK]
    unit_sizes = spans.sum(axis=1)                                     # [NU]
    offs = np.zeros(NU + 1, np.int64)
    np.cumsum(unit_sizes, out=offs[1:])
    GT = int(offs[-1])                        # total padded edges per core

    gidx = np.zeros((C, GT), np.int16)
    sidx = np.full((C, GT), DUSTBIN, np.int16)
    scale = np.zeros((C, GT), np.float32)

    starts = np.zeros((C, NU, NRANK), np.int64)
    flat = counts.reshape(-1)
    np.cumsum(flat[:-1], out=starts.reshape(-1)[1:])
    for c in range(C):
        for u in range(NU):
            pos = offs[u]
            for j in range(NRANK):
                s, n = starts[c, u, j], counts[c, u, j]
                gidx[c, pos : pos + n] = src_o[s : s + n]
                sidx[c, pos : pos + n] = sidx_o[s : s + n]
                scale[c, pos : pos + n] = scale_o[s : s + n]
                pos += spans[u, j]

    # wrapped layouts (per 16 / per 128 within each unit's region):
    #  idx tiles: [128, G/16] int16, idx i at [i%16, i//16], replicated x8 down
    #  scale tiles: [128, G/128] bf16, edge i at [i%128, i//128]
    gidx_w = np.zeros((C, 128, GT // 16), np.int16)
    sidx_w = np.zeros((C, 128, GT // 16), np.int16)
    scale_w = np.zeros((C, 128, GT // 128), bf16)
    for u in range(NU):
        o, g = offs[u], unit_sizes[u]
        wi = gidx[:, o : o + g].reshape(C, g // 16, 16).transpose(0, 2, 1)
        gidx_w[:, :, o // 16 : (o + g) // 16] = np.tile(wi, (1, 8, 1))
        wi = sidx[:, o : o + g].reshape(C, g // 16, 16).transpose(0, 2, 1)
        sidx_w[:, :, o // 16 : (o + g) // 16] = np.tile(wi, (1, 8, 1))
        ws = scale[:, o : o + g].reshape(C, g // 128, 128).transpose(0, 2, 1)
        scale_w[:, :, o // 128 : (o + g) // 128] = ws.astype(bf16)

    # replicated, block-padded x (bf16): [C*NLP, D]
    x = np.asarray(x, np.float32)
    x_rep = np.zeros((C * NLP, D), bf16)
    for c in range(C):
        x_rep[c * NLP : c * NLP + NL] = x[c * NL : (c + 1) * NL].astype(bf16)

    return dict(
        spans=tuple(tuple(int(v) for v in row if v) for row in spans),
        NU=NU,
        gidx=np.ascontiguousarray(gidx_w),
        sidx=np.ascontiguousarray(sidx_w),
        scale=np.ascontiguousarray(scale_w),
        x_rep=x_rep,
    )


# ----------------------------------------------------------------------------
# Device program
# ----------------------------------------------------------------------------
def build_program(cfg, spans, NU):
    N, E, D, R, C = cfg["N"], cfg["E"], cfg["D"], cfg["R"], cfg["C"]
    NL, NT, NLP, RP, TCH = derive(cfg)
    unit_sizes = [sum(row) for row in spans]
    offs = [0]
    for g in unit_sizes:
        offs.append(offs[-1] + g)
    GT = offs[-1]                        # total padded edges per core
    NCH = NT // TCH                      # transform chunks

    nc = bacc.Bacc(
        "TRN2", target_bir_lowering=False, debug=False,
        enable_asserts=False, num_devices=C,
    )

    # ---- I/O ----
    x_rep = nc.dram_tensor("x_rep", [C * NLP, D], BF16, kind="ExternalInput")
    x_loc = nc.dram_tensor("x_loc", [NLP, D], BF16, kind="ExternalInput")
    w_all = nc.dram_tensor("w_all", [2, R + 1, D, D], BF16, kind="ExternalInput")
    b_all = nc.dram_tensor("b_all", [2, 1, D], BF16, kind="ExternalInput")
    gidx_d = nc.dram_tensor("gidx", [128, GT // 16], I16, kind="ExternalInput")
    sidx_d = nc.dram_tensor("sidx", [128, GT // 16], I16, kind="ExternalInput")
    scale_d = nc.dram_tensor("scale", [128, GT // 128], BF16, kind="ExternalInput")

    # zero-initialized accumulators (ExternalOutputs are pre-zeroed)
    A = [
        [nc.dram_tensor(f"A_{l}_{p}", [2 * NLP, D], BF16, kind="ExternalOutput")
         for p in range(RP)]
        for l in range(2)
    ]
    out_d = nc.dram_tensor("out", [NL, D], F32, kind="ExternalOutput")

    # internal buffers for the inter-layer AllGather
    h1b = nc.dram_tensor("h1b", [NLP, D], BF16, kind="Internal")
    h1rep = nc.dram_tensor(
        "h1rep", [C * NLP, D], BF16, kind="Internal", addr_space="Shared"
    )

    with tile.TileContext(nc) as tc:
        with (
            tc.tile_pool(name="resident", bufs=1) as res_pool,
            tc.tile_pool(name="gather", bufs=3) as gpool,
            tc.tile_pool(name="station", bufs=2) as spool,
            tc.tile_pool(name="wpool", bufs=1) as wpool,
            tc.tile_pool(name="hout", bufs=4) as hpool,
            tc.tile_pool(name="psum", bufs=4, space="PSUM") as psum_pool,
        ):
            # resident index/scale tiles (shared by both layers)
            gidx_sb = res_pool.tile([128, GT // 16], I16)
            sidx_sb = res_pool.tile([128, GT // 16], I16)
            scale_sb = res_pool.tile([128, GT // 128], BF16)
            nc.sync.dma_start(out=gidx_sb[:], in_=gidx_d.ap()[:, :])
            nc.sync.dma_start(out=sidx_sb[:], in_=sidx_d.ap()[:, :])
            nc.sync.dma_start(out=scale_sb[:], in_=scale_d.ap()[:, :])

            ones_sb = res_pool.tile([1, D], BF16)
            nc.vector.memset(ones_sb[:], 1.0)

            for lay in range(2):
                src_tab = x_rep if lay == 0 else h1rep
                loc_tab = x_loc if lay == 0 else h1b

                # ---- edge phase: gather -> scale -> scatter-add ----
                es = nc.enter_named_scope(f"edge_{lay}", False)
                for u in range(NU):
                    blk, rpi = u // RP, u % RP
                    G = unit_sizes[u]
                    o = offs[u]
                    g = gpool.tile([128, G // 128, D], BF16, tag="g")
                    # HW caps one gather/scatter call at <2048 indices
                    for co in range(0, G, MAXC):
                        n = min(MAXC, G - co)
                        nc.gpsimd.dma_gather(
                            out_ap=g[:, co // 128 : (co + n) // 128, :],
                            in_ap=src_tab.ap()[blk * NLP : blk * NLP + NLP, :],
                            idxs_ap=gidx_sb[:, (o + co) // 16 : (o + co + n) // 16],
                            num_idxs=n,
                            num_idxs_reg=n,
                            elem_size=D,
                            single_packet=SINGLE_PACKET,
                        )
                    sc = scale_sb[:, o // 128 : (o + G) // 128]
                    nc.vector.tensor_tensor(
                        out=g[:],
                        in0=g[:],
                        in1=sc[:, :, None].to_broadcast([128, G // 128, D]),
                        op=mybir.AluOpType.mult,
                    )
                    # one scatter sub-call per duplicate-rank span piece:
                    # destination indices are unique within each call
                    so = 0
                    for span in spans[u]:
                        for po in range(0, span, MAXC):
                            n = min(MAXC, span - po)
                            p0 = so + po
                            nc.gpsimd.dma_scatter_add(
                                out_ap=A[lay][rpi].ap()[:, :],
                                in_ap=g[:, p0 // 128 : (p0 + n) // 128, :],
                                idxs_ap=sidx_sb[
                                    :, (o + p0) // 16 : (o + p0 + n) // 16
                                ],
                                num_idxs=n,
                                num_idxs_reg=n,
                                elem_size=D,
                                single_packet=SINGLE_PACKET,
                            )
                        so += span

                nc.leave_named_scope(f"edge_{lay}", es[0], False)
                ts = nc.enter_named_scope(f"transform_{lay}", False)
                # ---- weights for this layer ----
                w_sb = wpool.tile([128, (R + 1) * D], BF16, tag="w", bufs=2)
                nc.sync.dma_start(
                    out=w_sb[:].rearrange("d (r e) -> d r e", r=R + 1),
                    in_=w_all.ap()[lay].rearrange("r d e -> d r e"),
                )
                b_sb = wpool.tile([1, D], BF16, tag="b", bufs=2)
                nc.sync.dma_start(out=b_sb[:], in_=b_all.ap()[lay])

                # ---- transform phase ----
                for ch in range(NCH):
                    row0 = ch * TCH * 128
                    sts = []
                    for r in range(R):
                        st = spool.tile([128, TCH * 128], BF16, tag=f"st{r}")
                        a_rows = A[lay][r // 2].ap()[
                            (r % 2) * NLP + row0 : (r % 2) * NLP + row0 + TCH * 128, :
                        ]
                        nc.sync.dma_start_transpose(out=st[:], in_=a_rows)
                        sts.append(st)
                    st_x = spool.tile([128, TCH * 128], BF16, tag="stx")
                    nc.sync.dma_start_transpose(
                        out=st_x[:], in_=loc_tab.ap()[row0 : row0 + TCH * 128, :]
                    )

                    for t in range(TCH):
                        ps = psum_pool.tile([128, D], F32, tag="ps")
                        for r in range(R):
                            nc.tensor.matmul(
                                out=ps[:],
                                lhsT=sts[r][:, t * 128 : (t + 1) * 128],
                                rhs=w_sb[:, r * D : (r + 1) * D],
                                start=(r == 0),
                                stop=False,
                            )
                        nc.tensor.matmul(
                            out=ps[:],
                            lhsT=st_x[:, t * 128 : (t + 1) * 128],
                            rhs=w_sb[:, R * D : (R + 1) * D],
                            start=False,
                            stop=False,
                        )
                        nc.tensor.matmul(
                            out=ps[:],
                            lhsT=ones_sb[:1, :],
                            rhs=b_sb[:1, :],
                            start=False,
                            stop=True,
                        )
                        row = row0 + t * 128
                        if lay == 0:
                            hs = hpool.tile([128, D], BF16, tag="h0")
                            nc.scalar.activation(
                                out=hs[:], in_=ps[:],
                                func=mybir.ActivationFunctionType.Relu,
                            )
                            nc.sync.dma_start(
                                out=h1b.ap()[row : row + 128, :], in_=hs[:]
                            )
                        else:
                            nrow = min(128, NL - row)
                            if nrow <= 0:
                                continue
                            hs = hpool.tile([128, D], F32, tag="h1")
                            nc.scalar.activation(
                                out=hs[:], in_=ps[:],
                                func=mybir.ActivationFunctionType.Relu,
                            )
                            nc.sync.dma_start(
                                out=out_d.ap()[row : row + nrow, :],
                                in_=hs[:nrow, :],
                            )

                nc.leave_named_scope(f"transform_{lay}", ts[0], False)
                # ---- inter-layer AllGather (replicate new features) ----
                if lay == 0:
                    nc.gpsimd.collective_compute(
                        "AllGather",
                        mybir.AluOpType.bypass,
                        replica_groups=[list(range(C))],
                        ins=[h1b.ap()],
                        outs=[h1rep.ap()],
                    )

    nc.compile()
    return nc


# ----------------------------------------------------------------------------
# In-map assembly (shared by kernel() and tests)
# ----------------------------------------------------------------------------
def make_in_maps(prep, W1, root1, b1, W2, root2, b2, cfg):
    C, D, R = cfg["C"], cfg["D"], cfg["R"]
    NL, NT, NLP, RP, TCH = derive(cfg)
    w_all = np.zeros((2, R + 1, D, D), bf16)
    w_all[0, :R] = np.asarray(W1, np.float32).astype(bf16)
    w_all[0, R] = np.asarray(root1, np.float32).astype(bf16)
    w_all[1, :R] = np.asarray(W2, np.float32).astype(bf16)
    w_all[1, R] = np.asarray(root2, np.float32).astype(bf16)
    b_stack = np.stack([np.asarray(b1, np.float32), np.asarray(b2, np.float32)])
    b_all = b_stack.reshape(2, 1, D).astype(bf16)

    in_maps = []
    for c in range(C):
        x_loc = np.ascontiguousarray(prep["x_rep"][c * NLP : (c + 1) * NLP])
        in_maps.append({
            "x_rep": prep["x_rep"],
            "x_loc": x_loc,
            "w_all": w_all,
            "b_all": b_all,
            "gidx": prep["gidx"][c],
            "sidx": prep["sidx"][c],
            "scale": prep["scale"][c],
        })
    return in_maps


def enable_ntff_hook():
    """Register the axon NTFF profiling hook if the image's antenv lacks it."""
    import sys, types
    try:
        import antenv.axon_hooks  # noqa: F401
        return True
    except ImportError:
        pass
    try:
        from trn_agent_boot.trn_boot import _ntff_profile_via_ctypes
        hook = _ntff_profile_via_ctypes("/opt/axon/libaxon_pjrt.so")
        mod = types.ModuleType("antenv.axon_hooks")
        mod._hook = hook
        mod.set_axon_ntff_profile_hook = lambda h: setattr(mod, "_hook", h)
        mod.get_axon_ntff_profile_hook = lambda: mod._hook
        sys.modules["antenv.axon_hooks"] = mod
        import antenv
        antenv.axon_hooks = mod
        return hook is not None
    except Exception:
        return False


_program_cache = {}


def run(x, edge_index, edge_type, W1, root1, b1, W2, root2, b2,
        cfg=FULL, trace=False):
    prep = host_prep(x, edge_index, edge_type, cfg)
    key = (tuple(sorted(cfg.items())), prep["spans"], prep["NU"])
    if key not in _program_cache:
        _program_cache[key] = build_program(cfg, prep["spans"], prep["NU"])
    nc = _program_cache[key]
    in_maps = make_in_maps(prep, W1, root1, b1, W2, root2, b2, cfg)
    if trace:
        trace = enable_ntff_hook()
    res = run_bass_kernel_spmd(
        nc, in_maps, core_ids=list(range(cfg["C"])), trace=trace
    )
    blocks = [res.results[c]["out"] for c in range(cfg["C"])]
    full = np.concatenate(blocks, axis=0).astype(np.float32)
    return full, res


def kernel(**inputs):
    out, _ = run(
        inputs["x"], inputs["edge_index"], inputs["edge_type"],
        inputs["W1"], inputs["root1"], inputs["b1"],
        inputs["W2"], inputs["root2"], inputs["b2"],
    )
    return out

